# revision 71
# baseline (speedup 1.0000x reference)
"""Trainium2 Bass kernel for nn_MultiHeadAttention_59614146068609.

Sharding: 8 cores = 2 batches x 4 head-groups (4 heads each). Each core
projects q/k/v for its batch with its head-slice of Wq/Wk/Wv
(column-sharded), runs causal+padded attention for its 4 heads, and
applies its row-slice of Wo, producing a partial [D, S] fp16 output.
The host sums the 4 partials per batch and adds bo.

Schedule: single software-pipelined pass. Attention is ACT(exp)-paced,
so projection and output (Wo) matmul groups are injected as PE filler
between attention steps; the PE stays busy while the scalar engine
churns exp.

Key layout choices:
 - scores computed transposed (S.T[k, q], k on partitions); softmax
   denominator comes from an appended ones-column of V.
 - heads processed in pairs: qT/kT hold a head pair stacked on
   partitions (64+64); QK runs as two concurrent row-tiled matmuls
   (tile_position (0,0)/(64,0)), scores for the pair land in one
   2-bank PSUM tile and one ACTIVATE(exp) covers both heads.
 - key-padding folded into V: padded V rows are zeroed and the
   ones-column holds the valid mask, so exp needs no per-block bias
   and masked keys contribute exactly zero weight and zero denominator.

Specialized at build time on kb_cap = number of 128-wide key blocks
containing any unpadded key.
"""

import numpy as np

S = 2048
B = 2
D = 1024
H = 16
DK = 64
N_CORES = 8
GROUPS = N_CORES // B          # head groups per batch = 4
HPG = H // GROUPS              # heads per group = 4
OC = HPG * DK                  # per-core projected dim = 256
OT = OC // 128                 # head pairs per core = 2
IT = D // 128                  # contraction tiles = 8
SC = S // 512                  # sequence chunks of 512 = 4
KB = S // 128                  # k blocks of 128 = 16

_cache = {}


def _build_nc(kb_cap):
    import concourse.bacc as bacc
    import concourse.bass as bass
    import concourse.mybir as mybir
    import concourse.tile as tile
    from concourse import library_config

    F32 = mybir.dt.float32
    FP16 = mybir.dt.float16
    FP8 = mybir.dt.float8e4
    Exp = mybir.ActivationFunctionType.Exp
    PSUM = bass.MemorySpace.PSUM

    ksc = -(-kb_cap * 128 // 512)        # 512-chunks of k to project
    KW = ksc * 512
    VW = kb_cap * 128

    VC = -(-kb_cap // 4)                 # 512-wide chunks of v keys

    nc = bacc.Bacc("TRN2", target_bir_lowering=False, debug=False)

    # x streams pre-packed chunk-major on host: [128, chunk, IT, 512]
    xq = nc.dram_tensor("xq", [128, SC * IT * 512], FP16, kind="ExternalInput")
    xk = nc.dram_tensor("xk", [128, ksc * IT * 512], FP16, kind="ExternalInput")
    xv = nc.dram_tensor("xv", [128, VC * IT * 512], FP16, kind="ExternalInput")
    wq = nc.dram_tensor("wq", [128, IT * OC], FP16, kind="ExternalInput")
    wk = nc.dram_tensor("wk", [128, IT * OC], FP16, kind="ExternalInput")
    wv = nc.dram_tensor("wv", [128, IT * OC], FP16, kind="ExternalInput")
    wo = nc.dram_tensor("wo", [128, OT * D], FP16, kind="ExternalInput")
    # konst f32: [0:2]=bias_q(pair), [2:4]=bias_k, [4:20]=vmask, [20:276]=bv
    konst = nc.dram_tensor("konst", [128, 20 + OC], F32, kind="ExternalInput")
    # konst16 fp16: [0:64]=vmask4 (st-major), [64:320]=causal2, [320:448]=I
    konst16 = nc.dram_tensor("konst16", [128, KB * HPG + 256 + 128], FP16,
                             kind="ExternalInput")
    out_t = nc.dram_tensor("out_t", [D, S], FP16, kind="ExternalOutput")

    with tile.TileContext(nc) as tc, nc.allow_low_precision(
        reason="fp16 compute throughout; validated vs fp64 reference"
    ):
        with (
            tc.tile_pool(name="persist", bufs=1) as pp,
            tc.tile_pool(name="pt", bufs=3) as ptp,
            tc.tile_pool(name="nrm", bufs=2) as nrmp,
            tc.tile_pool(name="stg", bufs=3) as stgp,
            tc.tile_pool(name="vtmp", bufs=2) as vtp,
            tc.tile_pool(name="ps_st", bufs=2, space=PSUM) as ps_st,
            tc.tile_pool(name="ps_o", bufs=2, space=PSUM) as ps_o,
            tc.tile_pool(name="ps_w", bufs=2, space=PSUM) as ps_w,
        ):


            # ---- persistent SBUF tensors ----
            t_wq = pp.tile([128, IT, OC], FP16)
            t_wk = pp.tile([128, IT, OC], FP16)
            t_wv = pp.tile([128, IT, OC], FP16)
            t_wo = pp.tile([128, OT, D], FP16)
            t_k32 = pp.tile([128, 20 + OC], F32)
            t_k16 = pp.tile([128, KB * HPG + 256 + 128], FP16)
            t_qT = pp.tile([128, OT, S], FP16)
            t_kT = pp.tile([128, OT, KW], FP16)
            t_V = pp.tile([128, kb_cap, HPG, 128], FP16)
            t_OT = pp.tile([128, OT, S], FP16)

            t_xq = pp.tile([128, SC, IT, 512], FP16)
            t_xk = pp.tile([128, ksc, IT, 512], FP16)
            t_xv = pp.tile([128, VC, IT, 512], FP16)

            # ---- input DMAs; only SP/Activation/GpSimd queues can start DMAs.
            # Only the first chunk of each x stream moves upfront; later
            # chunks are enqueued mid-program (at flush points) so they don't
            # steal HBM bandwidth from the critical startup prefix.
            CW = IT * 512

            def load_x_chunk(eng, t_x, x_dram, c, split=1):
                ih = IT // split
                for h in range(split):
                    eng.dma_start(
                        out=t_x[:, c, h * ih:(h + 1) * ih, :],
                        in_=x_dram[:, c * CW + h * ih * 512:
                                   c * CW + (h + 1) * ih * 512].rearrange(
                            "p (i s) -> p i s", i=ih),
                    )

            # gpsimd's library load blocks its queue ~11us, so it carries no
            # startup DMAs; sync/scalar split the critical prefix in
            # need-order: q bundle and k bundle first, then the v bundle.
            nc.gpsimd.load_library(library_config.attn)
            nc.sync.dma_start(out=t_wq, in_=wq[:].rearrange("p (i o) -> p i o", i=IT))
            load_x_chunk(nc.sync, t_xq, xq, 0, split=2)
            nc.scalar.dma_start(out=t_k32, in_=konst[:])
            nc.scalar.dma_start(out=t_k16, in_=konst16[:])
            nc.scalar.dma_start(out=t_wk, in_=wk[:].rearrange("p (i o) -> p i o", i=IT))
            load_x_chunk(nc.scalar, t_xk, xk, 0, split=2)
            load_x_chunk(nc.sync, t_xv, xv, 0, split=2)
            nc.scalar.dma_start(out=t_wv, in_=wv[:].rearrange("p (i o) -> p i o", i=IT))

            def load_later_chunks(qc):
                # called at flush(qc): bring in the chunks needed next
                c = qc + 1
                if c < SC:
                    load_x_chunk(nc.sync, t_xq, xq, c)
                if c < ksc:
                    load_x_chunk(nc.scalar, t_xk, xk, c)
                if c < VC:
                    load_x_chunk(nc.scalar if c == 1 else nc.gpsimd, t_xv, xv, c)
                if c == 1:  # wo needed once C(0) fillers start popping
                    nc.gpsimd.dma_start(
                        out=t_wo, in_=wo[:].rearrange("p (j d) -> p j d", j=OT))

            # early dummy exp: pull the ACT table load into the startup window
            nc.scalar.activation(
                out=t_OT[0:1, 0, 0:1], in_=t_k32[0:1, 0:1], func=Exp)

            # HAM warmup: ~4.3us of dense dummy matmuls while the first x
            # chunks are still landing, so the PE clock gate opens (1.2 ->
            # 2.4 GHz) before the real projection stream begins
            warm_src = pp.tile([64, 128], FP16)
            nc.vector.memset(warm_src, 0.0)
            warm = ps_w.tile([128, 512], F32, tag="w", name="warm")
            for i in range(64):
                nc.tensor.matmul(
                    warm[:, 0:128], warm_src, warm_src, start=True, stop=True)

            # ---- work-unit generators ----
            def qk_proj_group(which, pair, sc):
                w_sb = t_wq if which == "q" else t_wk
                xts = t_xq if which == "q" else t_xk
                dst = t_qT if which == "q" else t_kT
                bidx = 0 if which == "q" else 1
                acc = ps_w.tile([128, 512], F32, tag="w",
                                name=f"acc_{which}_{pair}_{sc}")
                for i in range(IT):
                    nc.tensor.matmul(
                        acc,
                        w_sb[:, i, pair * 128:(pair + 1) * 128],
                        xts[:, sc, i, :],
                        start=(i == 0),
                        stop=(i == IT - 1),
                    )
                nc.vector.tensor_scalar_add(
                    out=dst[:, pair, sc * 512:(sc + 1) * 512],
                    in0=acc,
                    scalar1=t_k32[:, bidx * 2 + pair:bidx * 2 + pair + 1],
                )

            def v_proj_group(st):
                vacc = ps_w.tile([128, 512], F32, tag="w", name=f"vacc_{st}")
                for i in range(IT):
                    nc.tensor.matmul(
                        vacc[:, 0:OC],
                        t_xv[:, st // 4, i, (st % 4) * 128:(st % 4 + 1) * 128],
                        t_wv[:, i, :],
                        start=(i == 0),
                        stop=(i == IT - 1),
                    )
                tmp = vtp.tile([128, OC], F32, tag="vt", name=f"vt_{st}", bufs=2)
                nc.vector.tensor_add(out=tmp, in0=vacc[:, 0:OC], in1=t_k32[:, 20:20 + OC])
                nc.vector.tensor_scalar_mul(
                    out=t_V[:, st, :, 0:DK],
                    in0=tmp.rearrange("p (h d) -> p h d", h=HPG),
                    scalar1=t_k32[:, 4 + st:5 + st],
                )
                # ones-column of V = valid-key mask for this block
                nc.vector.tensor_copy(
                    t_V[:, st, :, DK:DK + 1],
                    t_k16[:, st * HPG:(st + 1) * HPG].rearrange(
                        "p (h o) -> p h o", o=1),
                )

            def c_group(qc, dt):
                q0 = qc * 512
                pc = ps_w.tile([128, 512], F32, tag="w", name=f"c_{qc}_{dt}")
                for j in range(OT):
                    nc.tensor.matmul(
                        pc,
                        t_wo[:, j, dt * 128:(dt + 1) * 128],
                        t_OT[:, j, q0:q0 + 512],
                        start=(j == 0),
                        stop=(j == OT - 1),
                    )
                so = stgp.tile([128, 512], FP16, tag="so", name=f"so_{qc}_{dt}")
                nc.vector.tensor_copy(so, pc)
                nc.sync.dma_start(
                    out=out_t[dt * 128:(dt + 1) * 128, q0:q0 + 512], in_=so)

            # filler queue: (deadline in (qc, pair) units, cost_estimate, fn);
            # attn(qc, pair) only needs its own pair's projections, so pair1
            # groups can pop as filler during pair0's attention
            fill = []
            for sc in range(SC):
                for pair in range(OT):
                    fill.append((sc, 1.7,
                                 lambda p=pair, s=sc: qk_proj_group("q", p, s)))
            for sc in range(ksc):
                for pair in range(OT):
                    fill.append((sc, 1.7,
                                 lambda p=pair, s=sc: qk_proj_group("k", p, s)))
            # v st<4 are issued inline inside attention(0, pair0) so the PE
            # can start QK/exp before the v x-chunk has landed
            n_inline_v = min(4, kb_cap)
            for st in range(n_inline_v, kb_cap):
                fill.append((st // 4, 0.9, lambda s=st: v_proj_group(s)))
            # order by deadline so flush/pacing pops prerequisites first
            fill.sort(key=lambda e: e[0])

            debt = [0.0]

            def maybe_fill(budget):
                debt[0] += budget
                while fill and debt[0] >= fill[0][1]:
                    _, cost, fn = fill.pop(0)
                    fn()
                    debt[0] -= cost

            def flush(u):
                while fill and fill[0][0] <= u:
                    _, _, fn = fill.pop(0)
                    fn()
                debt[0] = 0.0

            def attn_pair(qc, pair):
                q0 = qc * 512
                nkb = min(4 * (qc + 1), kb_cap)
                o_ps = [
                    ps_o.tile([128, 512], F32, tag="o", name=f"o_{qc}_{pair}_{a}")
                    for a in range(2)
                ]
                pts = {}

                def qk_exp(kb):
                    k0 = kb * 128
                    off = max(0, k0 - q0)
                    st = ps_st.tile([128, 1024], F32, tag="st",
                                    name=f"st_{qc}_{pair}_{kb}")
                    for a in range(2):
                        nc.tensor.matmul(
                            st[:, a * 512 + off:(a + 1) * 512],
                            t_kT[a * 64:(a + 1) * 64, pair, k0:k0 + 128],
                            t_qT[a * 64:(a + 1) * 64, pair, q0 + off:q0 + 512],
                            start=True,
                            stop=True,
                        )
                    if k0 >= q0:  # diagonal block: causal fix for both heads
                        for a in range(2):
                            nc.vector.tensor_add(
                                out=st[:, a * 512 + off:a * 512 + off + 128],
                                in0=st[:, a * 512 + off:a * 512 + off + 128],
                                in1=t_k16[:, KB * HPG:KB * HPG + 128],
                            )
                    pt = ptp.tile([128, 1024], FP16, tag="pt",
                                  name=f"pt_{qc}_{pair}_{kb}")
                    nc.scalar.activation(out=pt, in_=st, func=Exp)
                    pts[kb] = pt

                def pv(kb):
                    k0 = kb * 128
                    off = max(0, k0 - q0)
                    pt = pts.pop(kb)
                    for a in range(2):
                        nc.tensor.matmul(
                            o_ps[a][0:DK + 1, off:512],
                            t_V[:, kb, 2 * pair + a, 0:DK + 1],
                            pt[:, a * 512 + off:(a + 1) * 512],
                            start=(kb == 0),
                            stop=(kb == nkb - 1),
                        )

                inline_v = qc == 0 and pair == 0
                # pair0 of qc0 is fed by the inline v groups; its x-chunk is
                # still landing, so no regular pops there. Budgets taper so
                # some C groups survive to fill qc3's ACT-paced span.
                budget = 0.0 if inline_v else (0.56, 0.35, 0.35, 1.2)[qc]
                qk_exp(0)
                for kb in range(1, nkb):
                    qk_exp(kb)
                    if inline_v:
                        v_proj_group(kb - 1)
                    maybe_fill(budget)
                    pv(kb - 1)
                if inline_v:
                    v_proj_group(nkb - 1)
                maybe_fill(budget)
                pv(nkb - 1)

                for a in range(2):
                    # copy out of PSUM promptly (two base-0 pieces) so the
                    # o_ps bank frees for the next pair; norm runs from SBUF
                    t_l = nrmp.tile([1, 512], F32, tag="l",
                                    name=f"l_{qc}_{pair}_{a}", bufs=3)
                    nc.vector.tensor_copy(t_l, o_ps[a][DK:DK + 1, :])
                    o_sb = nrmp.tile([DK, 512], F32, tag="osb",
                                     name=f"osb_{qc}_{pair}_{a}", bufs=3)
                    nc.vector.tensor_copy(o_sb, o_ps[a][0:DK, :])
                    r = nrmp.tile([1, 512], F32, tag="r", name=f"r_{qc}_{pair}_{a}")
                    nc.vector.reciprocal_approx_fast(r, t_l)
                    rb = nrmp.tile([DK, 512], F32, tag="rb",
                                   name=f"rb_{qc}_{pair}_{a}")
                    nc.gpsimd.partition_broadcast(rb, r)
                    nc.vector.tensor_mul(
                        t_OT[a * 64:(a + 1) * 64, pair, q0:q0 + 512],
                        o_sb,
                        rb,
                    )

            # ---- main pipeline ----
            for qc in range(SC):
                load_later_chunks(qc)
                flush(qc)
                for pair in range(OT):
                    attn_pair(qc, pair)
                for dt in range(D // 128):
                    fill.append((SC + 1, 0.9, lambda q=qc, d=dt: c_group(q, d)))
            while fill:
                fill.pop(0)[2]()

    nc.compile()
    return nc


def _get_nc(kb_cap):
    key = ("nc", kb_cap)
    if key not in _cache:
        _cache[key] = _build_nc(kb_cap)
    return _cache[key]


def kernel(
    query,
    key,
    value,
    Wq,
    bq,
    Wk,
    bk,
    Wv,
    bv,
    Wo,
    bo,
    attn_mask,
    key_padding_mask,
):
    from concourse import bass_utils

    query = np.asarray(query, dtype=np.float32)
    key = np.asarray(key, dtype=np.float32)
    value = np.asarray(value, dtype=np.float32)
    Wq = np.asarray(Wq, dtype=np.float32)
    bq = np.asarray(bq, dtype=np.float32)
    Wk = np.asarray(Wk, dtype=np.float32)
    bk = np.asarray(bk, dtype=np.float32)
    Wv = np.asarray(Wv, dtype=np.float32)
    bv = np.asarray(bv, dtype=np.float32)
    Wo = np.asarray(Wo, dtype=np.float32)
    bo = np.asarray(bo, dtype=np.float32)
    attn_mask = np.asarray(attn_mask)
    key_padding_mask = np.asarray(key_padding_mask)

    # this kernel hardcodes the causal structure of attn_mask
    expected = np.triu(np.ones((S, S), dtype=bool), k=1)
    assert np.array_equal(attn_mask, expected), "kernel assumes causal attn_mask"

    # number of 128-blocks that contain any valid (unpadded) key
    valid = ~key_padding_mask  # [B, S]
    kb_cap = 0
    for b in range(B):
        nz = np.nonzero(valid[b])[0]
        cap = (int(nz.max()) // 128 + 1) if nz.size else 1
        kb_cap = max(kb_cap, cap)

    scale = np.float32(1.0 / np.sqrt(DK))
    ctile = np.where(
        np.arange(128)[None, :] >= np.arange(128)[:, None], 0.0, -60000.0
    ).astype(np.float16)
    causal2 = np.ascontiguousarray(np.concatenate([ctile, ctile], axis=1))

    def pack_w(w):  # [D, OC] -> [128, IT*OC] p-major
        return np.ascontiguousarray(
            w.reshape(IT, 128, OC).transpose(1, 0, 2).reshape(128, IT * OC)
        ).astype(np.float16)

    # per-batch transposed activations (shared by the batch's 4 cores),
    # packed chunk-major [128, chunks*IT*512] to match the SBUF layout
    ksc = -(-kb_cap * 128 // 512)
    VC = -(-kb_cap // 4)

    def pack_x(x, b, nchunks):  # x [S, B, D] -> [128, nchunks*IT*512]
        xt = x[:, b, :].T.astype(np.float16)  # [D, S]
        xt = xt[:, 0:nchunks * 512]
        return np.ascontiguousarray(
            xt.reshape(IT, 128, nchunks, 512).transpose(1, 2, 0, 3)
            .reshape(128, nchunks * IT * 512))

    xq_b = [pack_x(query, b, SC) for b in range(B)]
    xk_b = [pack_x(key, b, ksc) for b in range(B)]
    xv_b = [pack_x(value, b, VC) for b in range(B)]
    vm_b = [valid[b].astype(np.float32).reshape(KB, 128).T for b in range(B)]

    in_maps = []
    for c in range(N_CORES):
        b = c // GROUPS
        g = c % GROUPS
        o0 = g * OC
        osl = slice(o0, o0 + OC)
        konst = np.zeros((128, 20 + OC), np.float32)
        konst[:, 0:OT] = (bq[osl] * scale).reshape(OT, 128).T
        konst[:, 2:2 + OT] = bk[osl].reshape(OT, 128).T
        konst[:, 4:4 + KB] = vm_b[b]
        konst[:, 20:] = bv[osl][None, :]
        konst16 = np.zeros((128, KB * HPG + 256 + 128), np.float16)
        konst16[:, 0:KB * HPG] = np.repeat(
            vm_b[b].astype(np.float16)[:, :, None], HPG, axis=2
        ).reshape(128, KB * HPG)
        konst16[:, KB * HPG:KB * HPG + 256] = causal2
        konst16[:, KB * HPG + 256:] = np.eye(128, dtype=np.float16)
        in_maps.append(
            {
                "xq": xq_b[b],
                "xk": xk_b[b],
                "xv": xv_b[b],
                "wq": pack_w((Wq[osl, :] * scale).T),
                "wk": pack_w(Wk[osl, :].T),
                "wv": pack_w(Wv[osl, :].T),
                "wo": np.ascontiguousarray(
                    Wo[:, osl].T.reshape(OT, 128, D).transpose(1, 0, 2)
                    .reshape(128, OT * D)).astype(np.float16),
                "konst": np.ascontiguousarray(konst),
                "konst16": np.ascontiguousarray(konst16),
            }
        )

    res = bass_utils.run_bass_kernel_spmd(
        _get_nc(kb_cap), in_maps, core_ids=list(range(N_CORES))
    )
    _cache["last_res"] = res

    out = np.zeros((S, B, D), dtype=np.float32)
    for b in range(B):
        acc = np.zeros((D, S), dtype=np.float32)
        for g in range(GROUPS):
            acc += res.results[b * GROUPS + g]["out_t"].astype(np.float32)
        out[:, b, :] = acc.T + bo[None, :]
    return out
